# revision 3
# baseline (speedup 1.0000x reference)
"""Trainium2 Bass kernel for nn_CrossModalMoELayer (8 NeuronCores, SPMD).

Three SPMD launches + light host glue:
  Launch A (self-attn): core c = (batch b=c//2, head-group g=c%2).
    Each core computes its 8 heads' Q/K/V over all 512 tokens, attention,
    and the partial out-projection (contraction over its 512 head-dims),
    returning Y_part [512, 1024]. Host sums the pair + residual + bias,
    then applies the next LayerNorm (tiny) and re-chunks inputs.
  Launch B (cross-attn): same split, kv = image tokens (576).
  Host: gating softmax + expert-choice top-k routing, token gather.
  Launch C (MoE): core e = expert e; 2-layer gelu FFN over its 4*80 text
    + 4*90 image routed tokens (padded to 768), fp32r matmuls.
  Host: scatter-add expert outputs, divide by counts, add residuals.

On-chip layouts:
  token-major (tm): [tokens(part), feat(free)]
  feature-major chunked (fm): [128(part), kc*N + n]; chunk kc holds feature
    kc*128+p at partition p, free index n = token.
  Weights/activations in DRAM use "chunk_kc": X.T [K,M] -> [128, (K/128)*M],
    col = kc*M + m.
"""
import os
import sys
from contextlib import ExitStack

for _p in ('/opt/trn_rl_repo', '/root/.axon_site/_ro/trn_rl_repo'):
    if os.path.isdir(_p) and _p not in sys.path:
        sys.path.append(_p)

import numpy as np
import concourse.bass as bass
import concourse.tile as tile
from concourse import mybir
from concourse.bass_utils import run_bass_kernel_spmd
from concourse.vector_clock import ScopedClock
from concourse.masks import make_identity

F32 = mybir.dt.float32
F32R = mybir.dt.float32r
AF = mybir.ActivationFunctionType
ALU = mybir.AluOpType
AX = mybir.AxisListType

# problem dims
B, T, V_IMG, L = 4, 512, 576, 256
H, NH, I, E = 1024, 16, 4096, 8
GH = 512          # head-group width (8 heads x 64)
K_TXT, K_IMG = 80, 90
NTOK = 768        # padded tokens per expert (4*80+4*90=680)
EPS = 1e-5

TRACE = bool(os.environ.get("BASSK_TRACE"))
LAST_EXEC_NS = {}


class TC(tile.TileContext):
    """TileContext whose final drain splits sync waits one-per-instruction
    (this walrus build rejects >1 sync wait per instruction)."""

    def _drain_and_barrier(self, tick_clock, wait_clock):
        drain_inst = self.nc.sync.drain()
        wait_clock.add_sem_waits(
            drain_inst.ins, ScopedClock({None: tick_clock.global_clock}))
        si = drain_inst.ins.sync_info
        waits = list(si.on_wait) if si is not None else []
        if len(waits) > 1:
            si.on_wait = [waits[0]]
            for w in waits[1:]:
                d2 = self.nc.sync.drain()
                d2.ins.sync_info = mybir.SyncInfo(on_wait=[w], on_update=[])
        self.nc.all_engine_barrier()
        assert self.sems is not None
        popped = self.nc._tile_sem_poison_stack.pop()
        assert popped is self._sem_poison
        self.nc.clear_and_free_semaphores(list(self.sems.allocated().values()))
        self.nc.all_engine_barrier()


def split_multi_waits(nc):
    """Peel extra sync waits onto same-engine single-wait NoOps."""
    for fn in nc.m.functions:
        for bb in fn.blocks:
            new_insts = []
            for inst in bb.instructions:
                si = inst.sync_info
                if si is not None and si.on_wait and len(si.on_wait) > 1:
                    waits = list(si.on_wait)
                    for i, w in enumerate(waits[:-1]):
                        new_insts.append(mybir.InstNoOp(
                            name=f"{inst.name}-sw{i}",
                            engine=inst.engine,
                            sync_info=mybir.SyncInfo(on_wait=[w], on_update=[]),
                            bass_nofuse=True))
                    si.on_wait = [waits[-1]]
                new_insts.append(inst)
            bb.instructions[:] = new_insts


def _nsplits(n):
    out, off = [], 0
    while off < n:
        ln = min(512, n - off)
        out.append((off, ln))
        off += ln
    return out


def _bcast(ap, n):
    return bass.AP(tensor=ap.tensor, offset=ap.offset, ap=[[0, 128], [1, n]])


# ================= launches A/B: one attention head-group =================

def _emit_attn(tc, nc, d, nkv):
    nkc = (nkv + 127) // 128
    ctx = ExitStack()
    with ctx:
        const = ctx.enter_context(tc.tile_pool(name="const", bufs=1))
        big = ctx.enter_context(tc.tile_pool(name="big", bufs=1))
        wpool = ctx.enter_context(tc.tile_pool(name="wpool", bufs=2))
        work = ctx.enter_context(tc.tile_pool(name="work", bufs=2))
        ystage = ctx.enter_context(tc.tile_pool(name="ystage", bufs=3))
        small = ctx.enter_context(tc.tile_pool(name="small", bufs=6))
        psum = ctx.enter_context(tc.tile_pool(name="psum", bufs=2, space="PSUM"))
        psum_t = ctx.enter_context(tc.tile_pool(name="psum_t", bufs=2, space="PSUM"))
        psum_o = ctx.enter_context(tc.tile_pool(name="psum_o", bufs=2, space="PSUM"))

        ident = const.tile([128, 128], F32)
        make_identity(nc, ident)
        bias_t = const.tile([128, 12], F32)
        nc.sync.dma_start(bias_t[:, :], d['bqkv'][:, :])
        bv_t = const.tile([128, GH], F32)
        nc.sync.dma_start(bv_t[:, :], _bcast(d['bv_row'][:, :], GH))

        qfm = big.tile([128, 8 * T], F32)
        nc.sync.dma_start(qfm[:, :], d['qfm'][:, :])
        if nkv == T:
            kvfm = qfm
        else:
            kvfm = big.tile([128, 8 * nkv], F32)
            nc.sync.dma_start(kvfm[:, :], d['kvfm'][:, :])

        def load_w(dram, ncols):
            wt = wpool.tile([128, 4096], F32, tag="w")
            nc.sync.dma_start(wt[:, :ncols], dram[:, :ncols])
            return wt

        # --- Q/K projections: dst chunk mt = my-head features mt*128 ---
        def proj_fm(dst_fm, src_fm, n_src, wt, bias_col, scale_eighth):
            for mt in range(4):
                ps = psum.tile([128, 640], F32, tag="ps")
                for noff, nlen in _nsplits(n_src):
                    for kc in range(8):
                        nc.tensor.matmul(
                            ps[:, noff:noff + nlen],
                            wt[:, kc * GH + mt * 128: kc * GH + (mt + 1) * 128],
                            src_fm[:, kc * n_src + noff: kc * n_src + noff + nlen],
                            start=(kc == 0), stop=(kc == 7))
                if scale_eighth:
                    nc.vector.tensor_scalar(
                        dst_fm[:, mt * n_src:(mt + 1) * n_src], ps[:, :n_src],
                        bias_t[:, bias_col + mt: bias_col + mt + 1], 0.125,
                        op0=ALU.add, op1=ALU.mult)
                else:
                    nc.vector.tensor_scalar_add(
                        dst_fm[:, mt * n_src:(mt + 1) * n_src], ps[:, :n_src],
                        bias_t[:, bias_col + mt: bias_col + mt + 1])

        wt = load_w(d['wq'], 4096)
        Q_fm = big.tile([128, 4 * T], F32)
        proj_fm(Q_fm, qfm, T, wt, 0, True)
        wt = load_w(d['wk'], 4096)
        K_fm = big.tile([128, 4 * nkv], F32)
        proj_fm(K_fm, kvfm, nkv, wt, 4, False)

        # --- V projection (token-major, 512 out-features) ---
        wt = load_w(d['wv'], 4096)
        V_tm = big.tile([128, 5 * GH], F32)
        for tc_ in range(nkc):
            rows = min(128, nkv - tc_ * 128)
            ps = psum.tile([128, 640], F32, tag="ps")
            for kc in range(8):
                nc.tensor.matmul(
                    ps[:rows, :GH],
                    kvfm[:, kc * nkv + tc_ * 128: kc * nkv + tc_ * 128 + rows],
                    wt[:, kc * GH:(kc + 1) * GH],
                    start=(kc == 0), stop=(kc == 7))
            nc.vector.tensor_add(
                V_tm[:rows, tc_ * GH:(tc_ + 1) * GH], ps[:rows, :GH],
                bv_t[:rows, :])

        # --- attention: 8 heads (4 feature-chunks, 2 heads each) ---
        O_fm = big.tile([128, 4 * T], F32)
        for ft in range(4):
            po = psum_o.tile([128, T], F32, tag="po")
            for par in range(2):
                hoff = par * 64
                AT = work.tile([128, 5 * T], F32, tag="AT")
                for qc in range(4):
                    ps = psum.tile([128, 640], F32, tag="ps")
                    for noff, nlen in _nsplits(nkv):
                        nc.tensor.matmul(
                            ps[:, noff:noff + nlen],
                            Q_fm[hoff:hoff + 64,
                                 ft * T + qc * 128: ft * T + (qc + 1) * 128],
                            K_fm[hoff:hoff + 64,
                                 ft * nkv + noff: ft * nkv + noff + nlen],
                            start=True, stop=True)
                    smax = small.tile([128, 1], F32, tag="smax")
                    nc.vector.reduce_max(smax[:, :], ps[:, :nkv], axis=AX.X,
                                         negate=True)
                    A = work.tile([128, 640], F32, tag="A")
                    den = small.tile([128, 1], F32, tag="den")
                    nc.scalar.activation(A[:, :nkv], ps[:, :nkv], AF.Exp,
                                         bias=smax[:, :], scale=1.0,
                                         accum_out=den[:, :])
                    nc.vector.reciprocal(den[:, :], den[:, :])
                    nc.vector.tensor_scalar_mul(A[:, :nkv], A[:, :nkv],
                                                den[:, :])
                    for kt in range(nkc):
                        cl = min(128, nkv - kt * 128)
                        pt = psum_t.tile([128, 128], F32, tag="tp")
                        nc.tensor.matmul(pt[:cl, :], A[:, kt * 128: kt * 128 + cl],
                                         ident[:, :], is_transpose=True,
                                         start=True, stop=True)
                        nc.scalar.copy(
                            AT[:cl, kt * T + qc * 128: kt * T + (qc + 1) * 128],
                            pt[:cl, :])
                for kt in range(nkc):
                    rows = min(128, nkv - kt * 128)
                    nc.tensor.matmul(
                        po[hoff:hoff + 64, :],
                        V_tm[:rows, kt * GH + ft * 128 + hoff:
                             kt * GH + ft * 128 + hoff + 64],
                        AT[:rows, kt * T: (kt + 1) * T],
                        start=(kt == 0), stop=(kt == nkc - 1))
            nc.scalar.copy(O_fm[:, ft * T:(ft + 1) * T], po[:, :])

        # --- partial out-projection (contraction over my 512 features) ---
        wt = load_w(d['wo'], 4096)
        for tc_ in range(4):
            for half in range(2):
                ps = psum.tile([128, 640], F32, tag="ps")
                for kc in range(4):
                    nc.tensor.matmul(
                        ps[:, :512],
                        O_fm[:, kc * T + tc_ * 128: kc * T + (tc_ + 1) * 128],
                        wt[:, kc * 1024 + half * 512: kc * 1024 + (half + 1) * 512],
                        start=(kc == 0), stop=(kc == 3))
                yt = ystage.tile([128, 512], F32, tag="yt")
                nc.scalar.copy(yt[:, :], ps[:, :512])
                nc.sync.dma_start(
                    d['yp'][tc_ * 128:(tc_ + 1) * 128, half * 512:(half + 1) * 512],
                    yt[:, :])


def build_attn_nc(nkv):
    nc = bass.Bass("TRN2", target_bir_lowering=False, debug=False, num_devices=8)
    d = {}
    d['qfm'] = nc.dram_tensor("qfm", (128, 8 * T), F32, kind="ExternalInput")
    if nkv != T:
        d['kvfm'] = nc.dram_tensor("kvfm", (128, 8 * nkv), F32,
                                   kind="ExternalInput")
    d['wq'] = nc.dram_tensor("wq", (128, 8 * GH), F32, kind="ExternalInput")
    d['wk'] = nc.dram_tensor("wk", (128, 8 * GH), F32, kind="ExternalInput")
    d['wv'] = nc.dram_tensor("wv", (128, 8 * GH), F32, kind="ExternalInput")
    d['wo'] = nc.dram_tensor("wo", (128, 4 * H), F32, kind="ExternalInput")
    d['bqkv'] = nc.dram_tensor("bqkv", (128, 12), F32, kind="ExternalInput")
    d['bv_row'] = nc.dram_tensor("bv_row", (1, GH), F32, kind="ExternalInput")
    d['yp'] = nc.dram_tensor("yp", (T, H), F32, kind="ExternalOutput")
    with TC(nc) as tc:
        _emit_attn(tc, nc, d, nkv)
    split_multi_waits(nc)
    return nc


# ======================= launch C: expert FFN =======================

def _emit_moe(tc, nc, d):
    NSPLIT = ((0, 512), (512, 256))
    ctx = ExitStack()
    with ctx:
        const = ctx.enter_context(tc.tile_pool(name="const", bufs=1))
        xpool = ctx.enter_context(tc.tile_pool(name="xpool", bufs=1))
        hpool = ctx.enter_context(tc.tile_pool(name="hpool", bufs=1))
        w1pool = ctx.enter_context(tc.tile_pool(name="w1pool", bufs=3))
        w2pool = ctx.enter_context(tc.tile_pool(name="w2pool", bufs=2))
        ypool = ctx.enter_context(tc.tile_pool(name="ypool", bufs=2))
        psh = ctx.enter_context(tc.tile_pool(name="psh", bufs=2, space="PSUM"))
        psy = ctx.enter_context(tc.tile_pool(name="psy", bufs=2, space="PSUM"))

        b1_t = const.tile([128, 32], F32)
        nc.sync.dma_start(b1_t[:, :], d['b1'][:, :])
        b2_t = const.tile([128, 8], F32)
        nc.sync.dma_start(b2_t[:, :], d['b2'][:, :])
        x_t = xpool.tile([128, 8 * NTOK], F32R)
        nc.sync.dma_start(x_t[:, :], d['xfm'][:, :])
        h_t = hpool.tile([128, 32 * NTOK], F32R)

        for ic in range(32):
            wt = w1pool.tile([128, 8, 128], F32R, tag="w1t")
            nc.sync.dma_start(
                wt[:, :, :],
                d['w1'][:, :].rearrange("p (kc m) -> p kc m", kc=8)
                [:, :, ic * 128:(ic + 1) * 128])
            ph = psh.tile([128, NTOK], F32, tag="ph")
            for noff, nlen in NSPLIT:
                for kc in range(8):
                    nc.tensor.matmul(
                        ph[:, noff:noff + nlen], wt[:, kc, :],
                        x_t[:, kc * NTOK + noff: kc * NTOK + noff + nlen],
                        start=(kc == 0), stop=(kc == 7))
            nc.scalar.activation(h_t[:, ic * NTOK:(ic + 1) * NTOK], ph[:, :],
                                 AF.Gelu_apprx_tanh, bias=b1_t[:, ic:ic + 1],
                                 scale=1.0)

        for ft in range(8):
            wt = w2pool.tile([128, 32, 128], F32R, tag="w2t")
            nc.sync.dma_start(
                wt[:, :, :],
                d['w2'][:, :].rearrange("p (ic m) -> p ic m", ic=32)
                [:, :, ft * 128:(ft + 1) * 128])
            py = psy.tile([128, NTOK], F32, tag="py")
            for noff, nlen in NSPLIT:
                for ic in range(32):
                    nc.tensor.matmul(
                        py[:, noff:noff + nlen], wt[:, ic, :],
                        h_t[:, ic * NTOK + noff: ic * NTOK + noff + nlen],
                        start=(ic == 0), stop=(ic == 31))
            y_t = ypool.tile([128, NTOK], F32, tag="yt")
            nc.vector.tensor_scalar_add(y_t[:, :], py[:, :], b2_t[:, ft:ft + 1])
            nc.sync.dma_start(d['yfm'][:, ft * NTOK:(ft + 1) * NTOK], y_t[:, :])


def build_moe_nc():
    nc = bass.Bass("TRN2", target_bir_lowering=False, debug=False, num_devices=8)
    d = {}
    d['xfm'] = nc.dram_tensor("xfm", (128, 8 * NTOK), F32R, kind="ExternalInput")
    d['w1'] = nc.dram_tensor("w1", (128, 8 * I), F32R, kind="ExternalInput")
    d['w2'] = nc.dram_tensor("w2", (128, 32 * H), F32R, kind="ExternalInput")
    d['b1'] = nc.dram_tensor("b1", (128, 32), F32, kind="ExternalInput")
    d['b2'] = nc.dram_tensor("b2", (128, 8), F32, kind="ExternalInput")
    d['yfm'] = nc.dram_tensor("yfm", (128, 8 * NTOK), F32, kind="ExternalOutput")
    with TC(nc) as tc:
        _emit_moe(tc, nc, d)
    split_multi_waits(nc)
    return nc


# ======================= host-side helpers =======================

def chunk_kc(wt):
    K, M = wt.shape
    return np.ascontiguousarray(
        wt.reshape(K // 128, 128, M).transpose(1, 0, 2).reshape(128, -1))


def unchunk_kc(a, K):
    M = a.shape[1] // (K // 128)
    return a.reshape(128, K // 128, M).transpose(1, 0, 2).reshape(K, M)


def pp_bias(b):
    return np.ascontiguousarray(b.reshape(-1, 128).T)


def _f32(x):
    return np.ascontiguousarray(np.asarray(x, np.float32))


def _ln_host(x, g, b):
    x64 = x.astype(np.float64)
    m = x64.mean(-1, keepdims=True)
    v = x64.var(-1, keepdims=True)
    return (((x64 - m) / np.sqrt(v + EPS)) * g + b).astype(np.float32)


def _softmax_f32(x):
    x = x.astype(np.float32)
    m = x.max(-1, keepdims=True)
    e = np.exp(x - m, dtype=np.float32)
    return e / e.sum(-1, keepdims=True, dtype=np.float32)


def _topk_sets(probs, k):
    S, E_ = probs.shape
    out = np.empty((E_, k), np.int64)
    for e in range(E_):
        out[e] = np.argsort(-probs[:, e], kind='stable')[:k]
    return out


def _attn_w_maps(w_in, b_in, w_out):
    """Per-head-group weight dicts for one MHA layer."""
    wq, wk, wv = w_in[0:H], w_in[H:2 * H], w_in[2 * H:3 * H]
    maps = []
    for g in range(2):
        sl = slice(g * GH, (g + 1) * GH)
        maps.append(dict(
            wq=chunk_kc(np.ascontiguousarray(wq[sl].T)),
            wk=chunk_kc(np.ascontiguousarray(wk[sl].T)),
            wv=chunk_kc(np.ascontiguousarray(wv[sl].T)),
            wo=chunk_kc(np.ascontiguousarray(w_out[:, sl].T)),
            bqkv=np.concatenate([pp_bias(b_in[sl]), pp_bias(b_in[H:][sl]),
                                 pp_bias(b_in[2 * H:][sl])], axis=1),
            bv_row=np.ascontiguousarray(b_in[2 * H:][sl].reshape(1, -1))))
    return maps


_NC_CACHE = {}


def _get_nc(name):
    if name not in _NC_CACHE:
        if name == 'attn_sa':
            _NC_CACHE[name] = build_attn_nc(T)
        elif name == 'attn_ca':
            _NC_CACHE[name] = build_attn_nc(V_IMG)
        else:
            _NC_CACHE[name] = build_moe_nc()
    return _NC_CACHE[name]


def _run(nc, in_maps, label):
    kw = dict(trace=True) if TRACE else {}
    res = run_bass_kernel_spmd(nc, in_maps, core_ids=list(range(8)), **kw)
    if TRACE:
        LAST_EXEC_NS[label] = res.exec_time_ns
    return res.results


def _run_attn(name, x_fm_by_batch, kv_fm_by_batch, wmaps, label):
    nc = _get_nc(name)
    in_maps = []
    for c in range(8):
        b, g = c // 2, c % 2
        m = dict(qfm=x_fm_by_batch[b], **wmaps[g])
        if kv_fm_by_batch is not None:
            m['kvfm'] = kv_fm_by_batch[b]
        in_maps.append(m)
    res = _run(nc, in_maps, label)
    y = np.empty((B, T, H), np.float32)
    for b in range(B):
        y[b] = res[2 * b]['yp'] + res[2 * b + 1]['yp']
    return y


# ======================= top-level =======================

def kernel(**inputs):
    qt = _f32(inputs['query_tokens'])
    img = _f32(inputs['image_tokens'])
    txt = _f32(inputs['text_context'])

    # ---- launch A: self-attention ----
    qn = _ln_host(qt, _f32(inputs['lnq_g']), _f32(inputs['lnq_b']))
    qn_fm = [chunk_kc(np.ascontiguousarray(qn[b].T)) for b in range(B)]
    wm_sa = _attn_w_maps(_f32(inputs['sa_w_in']), _f32(inputs['sa_b_in']),
                         _f32(inputs['sa_w_out']))
    y_sa = _run_attn('attn_sa', qn_fm, None, wm_sa, 'sa')
    q1 = qt + y_sa + _f32(inputs['sa_b_out'])

    # ---- launch B: cross-attention ----
    cn = _ln_host(q1, _f32(inputs['lnc_g']), _f32(inputs['lnc_b']))
    cn_fm = [chunk_kc(np.ascontiguousarray(cn[b].T)) for b in range(B)]
    img_fm = [chunk_kc(np.ascontiguousarray(img[b].T)) for b in range(B)]
    wm_ca = _attn_w_maps(_f32(inputs['ca_w_in']), _f32(inputs['ca_b_in']),
                         _f32(inputs['ca_w_out']))
    y_ca = _run_attn('attn_ca', cn_fm, img_fm, wm_ca, 'ca')
    q2 = q1 + y_ca + _f32(inputs['ca_b_out'])

    # ---- host: LN3 + gating + routing ----
    ffn = _ln_host(q2, _f32(inputs['lnf_g']), _f32(inputs['lnf_b']))
    gate_img_w = _f32(inputs['gate_img_w']); gate_img_b = _f32(inputs['gate_img_b'])
    gate_txt_w = _f32(inputs['gate_txt_w']); gate_txt_b = _f32(inputs['gate_txt_b'])
    image_ctx = img.mean(1)
    text_ctx = txt.mean(1)
    tl = (q2 @ gate_txt_w[:, :H].T
          + (image_ctx @ gate_txt_w[:, H:].T + gate_txt_b)[:, None, :])
    il = (img @ gate_img_w[:, :H].T
          + (text_ctx @ gate_img_w[:, H:].T + gate_img_b)[:, None, :])
    text_probs = _softmax_f32(tl)
    image_probs = _softmax_f32(il)
    idx_t = np.stack([_topk_sets(text_probs[b], K_TXT) for b in range(B)])
    idx_i = np.stack([_topk_sets(image_probs[b], K_IMG) for b in range(B)])

    # ---- launch C: expert FFN ----
    e_w1 = _f32(inputs['e_w1']); e_b1 = _f32(inputs['e_b1'])
    e_w2 = _f32(inputs['e_w2']); e_b2 = _f32(inputs['e_b2'])
    in_maps2 = []
    for e in range(E):
        x = np.zeros((NTOK, H), np.float32)
        for b in range(B):
            x[b * K_TXT:(b + 1) * K_TXT] = ffn[b, idx_t[b, e]]
        for b in range(B):
            off = B * K_TXT + b * K_IMG
            x[off:off + K_IMG] = img[b, idx_i[b, e]]
        in_maps2.append(dict(
            xfm=chunk_kc(np.ascontiguousarray(x.T)),
            w1=chunk_kc(np.ascontiguousarray(e_w1[e].T)),
            w2=chunk_kc(np.ascontiguousarray(e_w2[e].T)),
            b1=pp_bias(e_b1[e]), b2=pp_bias(e_b2[e])))
    res2 = _run(_get_nc('moe'), in_maps2, 'moe')

    # ---- host: scatter-add + combine ----
    acc_q = np.zeros((B, T, H), np.float32)
    cnt_q = np.zeros((B, T), np.float32)
    acc_i = np.zeros((B, V_IMG, H), np.float32)
    cnt_i = np.zeros((B, V_IMG), np.float32)
    for e in range(E):
        y = unchunk_kc(res2[e]['yfm'], H).T
        for b in range(B):
            ids = idx_t[b, e]
            acc_q[b, ids] += y[b * K_TXT:(b + 1) * K_TXT]
            cnt_q[b, ids] += 1.0
            off = B * K_TXT + b * K_IMG
            ids = idx_i[b, e]
            acc_i[b, ids] += y[off:off + K_IMG]
            cnt_i[b, ids] += 1.0
    q_out = q2 + acc_q / np.maximum(cnt_q, 1.0)[..., None]
    img_out = img + acc_i / np.maximum(cnt_i, 1.0)[..., None]
    return q_out, img_out


# revision 4
# speedup vs baseline: 1.0350x; 1.0350x over previous
"""Trainium2 Bass kernel for nn_CrossModalMoELayer (8 NeuronCores, SPMD).

Three SPMD launches + light host glue:
  Launch A (self-attn): core c = (batch b=c//2, head-group g=c%2).
    Each core computes its 8 heads' Q/K/V over all 512 tokens, attention,
    and the partial out-projection (contraction over its 512 head-dims),
    returning Y_part [512, 1024]. Host sums the pair + residual + bias,
    then applies the next LayerNorm (tiny) and re-chunks inputs.
  Launch B (cross-attn): same split, kv = image tokens (576).
  Host: gating softmax + expert-choice top-k routing, token gather.
  Launch C (MoE): core e = expert e; 2-layer gelu FFN over its 4*80 text
    + 4*90 image routed tokens (padded to 768), fp32r matmuls.
  Host: scatter-add expert outputs, divide by counts, add residuals.

On-chip layouts:
  token-major (tm): [tokens(part), feat(free)]
  feature-major chunked (fm): [128(part), kc*N + n]; chunk kc holds feature
    kc*128+p at partition p, free index n = token.
  Weights/activations in DRAM use "chunk_kc": X.T [K,M] -> [128, (K/128)*M],
    col = kc*M + m.
"""
import os
import sys
from contextlib import ExitStack

for _p in ('/opt/trn_rl_repo', '/root/.axon_site/_ro/trn_rl_repo'):
    if os.path.isdir(_p) and _p not in sys.path:
        sys.path.append(_p)

import ml_dtypes
import numpy as np
import concourse.bass as bass
import concourse.tile as tile
from concourse import mybir
from concourse.bass_utils import run_bass_kernel_spmd
from concourse.vector_clock import ScopedClock
from concourse.masks import make_identity

F32 = mybir.dt.float32
F32R = mybir.dt.float32r
BF16 = mybir.dt.bfloat16
AF = mybir.ActivationFunctionType
ALU = mybir.AluOpType
AX = mybir.AxisListType

# problem dims
B, T, V_IMG, L = 4, 512, 576, 256
H, NH, I, E = 1024, 16, 4096, 8
GH = 512          # head-group width (8 heads x 64)
K_TXT, K_IMG = 80, 90
NTOK = 768        # padded tokens per expert (4*80+4*90=680)
EPS = 1e-5

TRACE = bool(os.environ.get("BASSK_TRACE"))
LAST_EXEC_NS = {}


class TC(tile.TileContext):
    """TileContext whose final drain splits sync waits one-per-instruction
    (this walrus build rejects >1 sync wait per instruction)."""

    def _drain_and_barrier(self, tick_clock, wait_clock):
        drain_inst = self.nc.sync.drain()
        wait_clock.add_sem_waits(
            drain_inst.ins, ScopedClock({None: tick_clock.global_clock}))
        si = drain_inst.ins.sync_info
        waits = list(si.on_wait) if si is not None else []
        if len(waits) > 1:
            si.on_wait = [waits[0]]
            for w in waits[1:]:
                d2 = self.nc.sync.drain()
                d2.ins.sync_info = mybir.SyncInfo(on_wait=[w], on_update=[])
        self.nc.all_engine_barrier()
        assert self.sems is not None
        popped = self.nc._tile_sem_poison_stack.pop()
        assert popped is self._sem_poison
        self.nc.clear_and_free_semaphores(list(self.sems.allocated().values()))
        self.nc.all_engine_barrier()


def split_multi_waits(nc):
    """Peel extra sync waits onto same-engine single-wait NoOps."""
    for fn in nc.m.functions:
        for bb in fn.blocks:
            new_insts = []
            for inst in bb.instructions:
                si = inst.sync_info
                if si is not None and si.on_wait and len(si.on_wait) > 1:
                    waits = list(si.on_wait)
                    for i, w in enumerate(waits[:-1]):
                        new_insts.append(mybir.InstNoOp(
                            name=f"{inst.name}-sw{i}",
                            engine=inst.engine,
                            sync_info=mybir.SyncInfo(on_wait=[w], on_update=[]),
                            bass_nofuse=True))
                    si.on_wait = [waits[-1]]
                new_insts.append(inst)
            bb.instructions[:] = new_insts


def _nsplits(n):
    out, off = [], 0
    while off < n:
        ln = min(512, n - off)
        out.append((off, ln))
        off += ln
    return out


def _bcast(ap, n):
    return bass.AP(tensor=ap.tensor, offset=ap.offset, ap=[[0, 128], [1, n]])


# ================= launches A/B: one attention head-group =================

def _emit_attn(tc, nc, d, nkv):
    nkc = (nkv + 127) // 128
    ctx = ExitStack()
    with ctx:
        const = ctx.enter_context(tc.tile_pool(name="const", bufs=1))
        big = ctx.enter_context(tc.tile_pool(name="big", bufs=1))
        wpool = ctx.enter_context(tc.tile_pool(name="wpool", bufs=2))
        work = ctx.enter_context(tc.tile_pool(name="work", bufs=2))
        ystage = ctx.enter_context(tc.tile_pool(name="ystage", bufs=3))
        small = ctx.enter_context(tc.tile_pool(name="small", bufs=6))
        psum = ctx.enter_context(tc.tile_pool(name="psum", bufs=2, space="PSUM"))
        psum_t = ctx.enter_context(tc.tile_pool(name="psum_t", bufs=2, space="PSUM"))
        psum_o = ctx.enter_context(tc.tile_pool(name="psum_o", bufs=2, space="PSUM"))

        ident = const.tile([128, 128], F32)
        make_identity(nc, ident)
        bias_t = const.tile([128, 12], F32)
        nc.sync.dma_start(bias_t[:, :], d['bqkv'][:, :])
        bv_t = const.tile([128, GH], F32)
        nc.sync.dma_start(bv_t[:, :], _bcast(d['bv_row'][:, :], GH))

        qfm = big.tile([128, 8 * T], F32)
        nc.sync.dma_start(qfm[:, :], d['qfm'][:, :])
        if nkv == T:
            kvfm = qfm
        else:
            kvfm = big.tile([128, 8 * nkv], F32)
            nc.sync.dma_start(kvfm[:, :], d['kvfm'][:, :])

        def load_w(dram, ncols):
            wt = wpool.tile([128, 4096], F32, tag="w")
            nc.sync.dma_start(wt[:, :ncols], dram[:, :ncols])
            return wt

        # --- Q/K projections: dst chunk mt = my-head features mt*128 ---
        def proj_fm(dst_fm, src_fm, n_src, wt, bias_col, scale_eighth):
            for mt in range(4):
                ps = psum.tile([128, 640], F32, tag="ps")
                for noff, nlen in _nsplits(n_src):
                    for kc in range(8):
                        nc.tensor.matmul(
                            ps[:, noff:noff + nlen],
                            wt[:, kc * GH + mt * 128: kc * GH + (mt + 1) * 128],
                            src_fm[:, kc * n_src + noff: kc * n_src + noff + nlen],
                            start=(kc == 0), stop=(kc == 7))
                if scale_eighth:
                    nc.vector.tensor_scalar(
                        dst_fm[:, mt * n_src:(mt + 1) * n_src], ps[:, :n_src],
                        bias_t[:, bias_col + mt: bias_col + mt + 1], 0.125,
                        op0=ALU.add, op1=ALU.mult)
                else:
                    nc.vector.tensor_scalar_add(
                        dst_fm[:, mt * n_src:(mt + 1) * n_src], ps[:, :n_src],
                        bias_t[:, bias_col + mt: bias_col + mt + 1])

        wt = load_w(d['wq'], 4096)
        Q_fm = big.tile([128, 4 * T], F32)
        proj_fm(Q_fm, qfm, T, wt, 0, True)
        wt = load_w(d['wk'], 4096)
        K_fm = big.tile([128, 4 * nkv], F32)
        proj_fm(K_fm, kvfm, nkv, wt, 4, False)

        # --- V projection (token-major, 512 out-features) ---
        wt = load_w(d['wv'], 4096)
        V_tm = big.tile([128, 5 * GH], F32)
        for tc_ in range(nkc):
            rows = min(128, nkv - tc_ * 128)
            ps = psum.tile([128, 640], F32, tag="ps")
            for kc in range(8):
                nc.tensor.matmul(
                    ps[:rows, :GH],
                    kvfm[:, kc * nkv + tc_ * 128: kc * nkv + tc_ * 128 + rows],
                    wt[:, kc * GH:(kc + 1) * GH],
                    start=(kc == 0), stop=(kc == 7))
            nc.vector.tensor_add(
                V_tm[:rows, tc_ * GH:(tc_ + 1) * GH], ps[:rows, :GH],
                bv_t[:rows, :])

        # --- attention: 8 heads (4 feature-chunks, 2 heads each) ---
        O_fm = big.tile([128, 4 * T], F32)
        for ft in range(4):
            po = psum_o.tile([128, T], F32, tag="po")
            for par in range(2):
                hoff = par * 64
                AT = work.tile([128, 5 * T], F32, tag="AT")
                for qc in range(4):
                    ps = psum.tile([128, 640], F32, tag="ps")
                    for noff, nlen in _nsplits(nkv):
                        nc.tensor.matmul(
                            ps[:, noff:noff + nlen],
                            Q_fm[hoff:hoff + 64,
                                 ft * T + qc * 128: ft * T + (qc + 1) * 128],
                            K_fm[hoff:hoff + 64,
                                 ft * nkv + noff: ft * nkv + noff + nlen],
                            start=True, stop=True)
                    smax = small.tile([128, 1], F32, tag="smax")
                    nc.vector.reduce_max(smax[:, :], ps[:, :nkv], axis=AX.X,
                                         negate=True)
                    A = work.tile([128, 640], F32, tag="A")
                    den = small.tile([128, 1], F32, tag="den")
                    nc.scalar.activation(A[:, :nkv], ps[:, :nkv], AF.Exp,
                                         bias=smax[:, :], scale=1.0,
                                         accum_out=den[:, :])
                    nc.vector.reciprocal(den[:, :], den[:, :])
                    nc.vector.tensor_scalar_mul(A[:, :nkv], A[:, :nkv],
                                                den[:, :])
                    for kt in range(nkc):
                        cl = min(128, nkv - kt * 128)
                        pt = psum_t.tile([128, 128], F32, tag="tp")
                        nc.tensor.matmul(pt[:cl, :], A[:, kt * 128: kt * 128 + cl],
                                         ident[:, :], is_transpose=True,
                                         start=True, stop=True)
                        nc.scalar.copy(
                            AT[:cl, kt * T + qc * 128: kt * T + (qc + 1) * 128],
                            pt[:cl, :])
                for kt in range(nkc):
                    rows = min(128, nkv - kt * 128)
                    nc.tensor.matmul(
                        po[hoff:hoff + 64, :],
                        V_tm[:rows, kt * GH + ft * 128 + hoff:
                             kt * GH + ft * 128 + hoff + 64],
                        AT[:rows, kt * T: (kt + 1) * T],
                        start=(kt == 0), stop=(kt == nkc - 1))
            nc.scalar.copy(O_fm[:, ft * T:(ft + 1) * T], po[:, :])

        # --- partial out-projection (contraction over my 512 features) ---
        wt = load_w(d['wo'], 4096)
        for tc_ in range(4):
            for half in range(2):
                ps = psum.tile([128, 640], F32, tag="ps")
                for kc in range(4):
                    nc.tensor.matmul(
                        ps[:, :512],
                        O_fm[:, kc * T + tc_ * 128: kc * T + (tc_ + 1) * 128],
                        wt[:, kc * 1024 + half * 512: kc * 1024 + (half + 1) * 512],
                        start=(kc == 0), stop=(kc == 3))
                yt = ystage.tile([128, 512], F32, tag="yt")
                nc.scalar.copy(yt[:, :], ps[:, :512])
                nc.sync.dma_start(
                    d['yp'][tc_ * 128:(tc_ + 1) * 128, half * 512:(half + 1) * 512],
                    yt[:, :])


def build_attn_nc(nkv):
    nc = bass.Bass("TRN2", target_bir_lowering=False, debug=False, num_devices=8)
    d = {}
    d['qfm'] = nc.dram_tensor("qfm", (128, 8 * T), F32, kind="ExternalInput")
    if nkv != T:
        d['kvfm'] = nc.dram_tensor("kvfm", (128, 8 * nkv), F32,
                                   kind="ExternalInput")
    d['wq'] = nc.dram_tensor("wq", (128, 8 * GH), F32, kind="ExternalInput")
    d['wk'] = nc.dram_tensor("wk", (128, 8 * GH), F32, kind="ExternalInput")
    d['wv'] = nc.dram_tensor("wv", (128, 8 * GH), F32, kind="ExternalInput")
    d['wo'] = nc.dram_tensor("wo", (128, 4 * H), F32, kind="ExternalInput")
    d['bqkv'] = nc.dram_tensor("bqkv", (128, 12), F32, kind="ExternalInput")
    d['bv_row'] = nc.dram_tensor("bv_row", (1, GH), F32, kind="ExternalInput")
    d['yp'] = nc.dram_tensor("yp", (T, H), F32, kind="ExternalOutput")
    with TC(nc) as tc:
        _emit_attn(tc, nc, d, nkv)
    split_multi_waits(nc)
    return nc


# ======================= launch C: expert FFN =======================

def _emit_moe(tc, nc, d):
    NSPLIT = ((0, 512), (512, 256))
    ctx = ExitStack()
    with ctx:
        const = ctx.enter_context(tc.tile_pool(name="const", bufs=1))
        xpool = ctx.enter_context(tc.tile_pool(name="xpool", bufs=1))
        hpool = ctx.enter_context(tc.tile_pool(name="hpool", bufs=1))
        w1pool = ctx.enter_context(tc.tile_pool(name="w1pool", bufs=3))
        w2pool = ctx.enter_context(tc.tile_pool(name="w2pool", bufs=2))
        ypool = ctx.enter_context(tc.tile_pool(name="ypool", bufs=2))
        psh = ctx.enter_context(tc.tile_pool(name="psh", bufs=2, space="PSUM"))
        psy = ctx.enter_context(tc.tile_pool(name="psy", bufs=2, space="PSUM"))

        b1_t = const.tile([128, 32], F32)
        nc.sync.dma_start(b1_t[:, :], d['b1'][:, :])
        b2_t = const.tile([128, 8], F32)
        nc.sync.dma_start(b2_t[:, :], d['b2'][:, :])
        x_t = xpool.tile([128, 8 * NTOK], BF16)
        nc.sync.dma_start(x_t[:, :], d['xfm'][:, :])
        h_t = hpool.tile([128, 32 * NTOK], BF16)

        for ic in range(32):
            wt = w1pool.tile([128, 8, 128], BF16, tag="w1t")
            nc.sync.dma_start(
                wt[:, :, :],
                d['w1'][:, :].rearrange("p (kc m) -> p kc m", kc=8)
                [:, :, ic * 128:(ic + 1) * 128])
            ph = psh.tile([128, NTOK], F32, tag="ph")
            for noff, nlen in NSPLIT:
                for kc in range(8):
                    nc.tensor.matmul(
                        ph[:, noff:noff + nlen], wt[:, kc, :],
                        x_t[:, kc * NTOK + noff: kc * NTOK + noff + nlen],
                        start=(kc == 0), stop=(kc == 7))
            nc.scalar.activation(h_t[:, ic * NTOK:(ic + 1) * NTOK], ph[:, :],
                                 AF.Gelu_apprx_tanh, bias=b1_t[:, ic:ic + 1],
                                 scale=1.0)

        for ft in range(8):
            wt = w2pool.tile([128, 32, 128], BF16, tag="w2t")
            nc.sync.dma_start(
                wt[:, :, :],
                d['w2'][:, :].rearrange("p (ic m) -> p ic m", ic=32)
                [:, :, ft * 128:(ft + 1) * 128])
            py = psy.tile([128, NTOK], F32, tag="py")
            for noff, nlen in NSPLIT:
                for ic in range(32):
                    nc.tensor.matmul(
                        py[:, noff:noff + nlen], wt[:, ic, :],
                        h_t[:, ic * NTOK + noff: ic * NTOK + noff + nlen],
                        start=(ic == 0), stop=(ic == 31))
            y_t = ypool.tile([128, NTOK], F32, tag="yt")
            nc.vector.tensor_scalar_add(y_t[:, :], py[:, :], b2_t[:, ft:ft + 1])
            nc.sync.dma_start(d['yfm'][:, ft * NTOK:(ft + 1) * NTOK], y_t[:, :])


def build_moe_nc():
    nc = bass.Bass("TRN2", target_bir_lowering=False, debug=False, num_devices=8)
    d = {}
    d['xfm'] = nc.dram_tensor("xfm", (128, 8 * NTOK), BF16, kind="ExternalInput")
    d['w1'] = nc.dram_tensor("w1", (128, 8 * I), BF16, kind="ExternalInput")
    d['w2'] = nc.dram_tensor("w2", (128, 32 * H), BF16, kind="ExternalInput")
    d['b1'] = nc.dram_tensor("b1", (128, 32), F32, kind="ExternalInput")
    d['b2'] = nc.dram_tensor("b2", (128, 8), F32, kind="ExternalInput")
    d['yfm'] = nc.dram_tensor("yfm", (128, 8 * NTOK), F32, kind="ExternalOutput")
    with TC(nc) as tc:
        _emit_moe(tc, nc, d)
    split_multi_waits(nc)
    return nc


# ======================= host-side helpers =======================

def chunk_kc(wt):
    K, M = wt.shape
    return np.ascontiguousarray(
        wt.reshape(K // 128, 128, M).transpose(1, 0, 2).reshape(128, -1))


def unchunk_kc(a, K):
    M = a.shape[1] // (K // 128)
    return a.reshape(128, K // 128, M).transpose(1, 0, 2).reshape(K, M)


def pp_bias(b):
    return np.ascontiguousarray(b.reshape(-1, 128).T)


def _f32(x):
    return np.ascontiguousarray(np.asarray(x, np.float32))


def _ln_host(x, g, b):
    x64 = x.astype(np.float64)
    m = x64.mean(-1, keepdims=True)
    v = x64.var(-1, keepdims=True)
    return (((x64 - m) / np.sqrt(v + EPS)) * g + b).astype(np.float32)


def _softmax_f32(x):
    x = x.astype(np.float32)
    m = x.max(-1, keepdims=True)
    e = np.exp(x - m, dtype=np.float32)
    return e / e.sum(-1, keepdims=True, dtype=np.float32)


def _topk_sets(probs, k):
    S, E_ = probs.shape
    out = np.empty((E_, k), np.int64)
    for e in range(E_):
        out[e] = np.argsort(-probs[:, e], kind='stable')[:k]
    return out


def _attn_w_maps(w_in, b_in, w_out):
    """Per-head-group weight dicts for one MHA layer."""
    wq, wk, wv = w_in[0:H], w_in[H:2 * H], w_in[2 * H:3 * H]
    maps = []
    for g in range(2):
        sl = slice(g * GH, (g + 1) * GH)
        maps.append(dict(
            wq=chunk_kc(np.ascontiguousarray(wq[sl].T)),
            wk=chunk_kc(np.ascontiguousarray(wk[sl].T)),
            wv=chunk_kc(np.ascontiguousarray(wv[sl].T)),
            wo=chunk_kc(np.ascontiguousarray(w_out[:, sl].T)),
            bqkv=np.concatenate([pp_bias(b_in[sl]), pp_bias(b_in[H:][sl]),
                                 pp_bias(b_in[2 * H:][sl])], axis=1),
            bv_row=np.ascontiguousarray(b_in[2 * H:][sl].reshape(1, -1))))
    return maps


_NC_CACHE = {}


def _get_nc(name):
    if name not in _NC_CACHE:
        if name == 'attn_sa':
            _NC_CACHE[name] = build_attn_nc(T)
        elif name == 'attn_ca':
            _NC_CACHE[name] = build_attn_nc(V_IMG)
        else:
            _NC_CACHE[name] = build_moe_nc()
    return _NC_CACHE[name]


def _run(nc, in_maps, label):
    kw = dict(trace=True) if TRACE else {}
    res = run_bass_kernel_spmd(nc, in_maps, core_ids=list(range(8)), **kw)
    if TRACE:
        LAST_EXEC_NS[label] = res.exec_time_ns
    return res.results


def _run_attn(name, x_fm_by_batch, kv_fm_by_batch, wmaps, label):
    nc = _get_nc(name)
    in_maps = []
    for c in range(8):
        b, g = c // 2, c % 2
        m = dict(qfm=x_fm_by_batch[b], **wmaps[g])
        if kv_fm_by_batch is not None:
            m['kvfm'] = kv_fm_by_batch[b]
        in_maps.append(m)
    res = _run(nc, in_maps, label)
    y = np.empty((B, T, H), np.float32)
    for b in range(B):
        y[b] = res[2 * b]['yp'] + res[2 * b + 1]['yp']
    return y


# ======================= top-level =======================

def kernel(**inputs):
    qt = _f32(inputs['query_tokens'])
    img = _f32(inputs['image_tokens'])
    txt = _f32(inputs['text_context'])

    # ---- launch A: self-attention ----
    qn = _ln_host(qt, _f32(inputs['lnq_g']), _f32(inputs['lnq_b']))
    qn_fm = [chunk_kc(np.ascontiguousarray(qn[b].T)) for b in range(B)]
    wm_sa = _attn_w_maps(_f32(inputs['sa_w_in']), _f32(inputs['sa_b_in']),
                         _f32(inputs['sa_w_out']))
    y_sa = _run_attn('attn_sa', qn_fm, None, wm_sa, 'sa')
    q1 = qt + y_sa + _f32(inputs['sa_b_out'])

    # ---- launch B: cross-attention ----
    cn = _ln_host(q1, _f32(inputs['lnc_g']), _f32(inputs['lnc_b']))
    cn_fm = [chunk_kc(np.ascontiguousarray(cn[b].T)) for b in range(B)]
    img_fm = [chunk_kc(np.ascontiguousarray(img[b].T)) for b in range(B)]
    wm_ca = _attn_w_maps(_f32(inputs['ca_w_in']), _f32(inputs['ca_b_in']),
                         _f32(inputs['ca_w_out']))
    y_ca = _run_attn('attn_ca', cn_fm, img_fm, wm_ca, 'ca')
    q2 = q1 + y_ca + _f32(inputs['ca_b_out'])

    # ---- host: LN3 + gating + routing ----
    ffn = _ln_host(q2, _f32(inputs['lnf_g']), _f32(inputs['lnf_b']))
    gate_img_w = _f32(inputs['gate_img_w']); gate_img_b = _f32(inputs['gate_img_b'])
    gate_txt_w = _f32(inputs['gate_txt_w']); gate_txt_b = _f32(inputs['gate_txt_b'])
    image_ctx = img.mean(1)
    text_ctx = txt.mean(1)
    tl = (q2 @ gate_txt_w[:, :H].T
          + (image_ctx @ gate_txt_w[:, H:].T + gate_txt_b)[:, None, :])
    il = (img @ gate_img_w[:, :H].T
          + (text_ctx @ gate_img_w[:, H:].T + gate_img_b)[:, None, :])
    text_probs = _softmax_f32(tl)
    image_probs = _softmax_f32(il)
    idx_t = np.stack([_topk_sets(text_probs[b], K_TXT) for b in range(B)])
    idx_i = np.stack([_topk_sets(image_probs[b], K_IMG) for b in range(B)])

    # ---- launch C: expert FFN ----
    e_w1 = _f32(inputs['e_w1']); e_b1 = _f32(inputs['e_b1'])
    e_w2 = _f32(inputs['e_w2']); e_b2 = _f32(inputs['e_b2'])
    in_maps2 = []
    for e in range(E):
        x = np.zeros((NTOK, H), np.float32)
        for b in range(B):
            x[b * K_TXT:(b + 1) * K_TXT] = ffn[b, idx_t[b, e]]
        for b in range(B):
            off = B * K_TXT + b * K_IMG
            x[off:off + K_IMG] = img[b, idx_i[b, e]]
        in_maps2.append(dict(
            xfm=chunk_kc(np.ascontiguousarray(x.T)).astype(ml_dtypes.bfloat16),
            w1=chunk_kc(np.ascontiguousarray(e_w1[e].T)).astype(ml_dtypes.bfloat16),
            w2=chunk_kc(np.ascontiguousarray(e_w2[e].T)).astype(ml_dtypes.bfloat16),
            b1=pp_bias(e_b1[e]), b2=pp_bias(e_b2[e])))
    res2 = _run(_get_nc('moe'), in_maps2, 'moe')

    # ---- host: scatter-add + combine ----
    acc_q = np.zeros((B, T, H), np.float32)
    cnt_q = np.zeros((B, T), np.float32)
    acc_i = np.zeros((B, V_IMG, H), np.float32)
    cnt_i = np.zeros((B, V_IMG), np.float32)
    for e in range(E):
        y = unchunk_kc(res2[e]['yfm'], H).T
        for b in range(B):
            ids = idx_t[b, e]
            acc_q[b, ids] += y[b * K_TXT:(b + 1) * K_TXT]
            cnt_q[b, ids] += 1.0
            off = B * K_TXT + b * K_IMG
            ids = idx_i[b, e]
            acc_i[b, ids] += y[off:off + K_IMG]
            cnt_i[b, ids] += 1.0
    q_out = q2 + acc_q / np.maximum(cnt_q, 1.0)[..., None]
    img_out = img + acc_i / np.maximum(cnt_i, 1.0)[..., None]
    return q_out, img_out
